# revision 13
# baseline (speedup 1.0000x reference)
"""StyleGAN2-style modulated conv (per-sample 3x3, 256->256 ch, 128x128) on 8 TRN2 cores.

Data-parallel over batch: core c computes sample c entirely on-chip.
1D Winograd F(2,3) along H cuts PE work 1.5x vs direct shift-and-matmul:
per output row-pair only 4 H-taps x 3 W-taps of N=512 matmuls (vs 2x9).

v2.2 structure:
- weight DMA split into one tile per (ot, it) chunk; PE transposes start
  per-chunk (dep tracking is tile-granular).
- style modulation folded into the transpose PSUM->SBUF evictions
  (per-partition scale on the copy), so wm is pre-modulated fp16 and the
  winograd weight build is 3 DVE ops per i-half (taps 0/3 alias wm).
- x DMAs land in zero-padded fp32 stage tiles; the DVE row transform
  reads fp32 and writes fp16 directly — no separate cast pass.
- wsq squares on ACT (scale=2 trick gives the x4 for half-scaled kh1),
  accumulation adds on GpSimd; denominator contracts against ones on PE.
- group 0 evicts raw with deferred demod (dn not on the startup critical
  path); steady-state groups fuse dn into the eviction (ACT scaled
  copies + DVE scalar_tensor_tensor), eliminating the demod pass.
"""

import numpy as np
from contextlib import ExitStack

import concourse.bass as bass
import concourse.mybir as mybir
import concourse.tile as tile
from concourse import bacc
from concourse.masks import make_identity

FP32 = mybir.dt.float32
FP16 = mybir.dt.float16
AX = mybir.AxisListType
AF = mybir.ActivationFunctionType
OP = mybir.AluOpType

B = 8
CI = 256
CO = 256
H = 128
W = 128
KS = 3
Z = 512
NKK = KS * KS          # 9 kernel taps
IT = CI // 128         # 2 input-channel tiles
OT = CO // 128         # 2 output-channel tiles
RG = 16                # output rows per group
G = H // RG            # 8 row groups
NP = RG // 2           # 8 winograd row-pairs per group
WP = W + 2             # padded width (zero cols at 0 and WP-1)
TH = 4                 # winograd taps along H
KH = KS * CO           # 768: one kh-slab (3 kw x 256 o)
EPS = 1e-8


def build_nc() -> bass.Bass:
    nc = bacc.Bacc("TRN2", target_bir_lowering=False, debug=False)
    x_d = nc.dram_tensor("x", [CI, H, W], FP32, kind="ExternalInput")
    w_d = nc.dram_tensor("w", [Z], FP32, kind="ExternalInput")
    wt_d = nc.dram_tensor("weight", [CO, CI, KS, KS], FP32, kind="ExternalInput")
    aw_d = nc.dram_tensor("affine_w", [CI, Z], FP32, kind="ExternalInput")
    ab_d = nc.dram_tensor("affine_b", [CI], FP32, kind="ExternalInput")
    y_d = nc.dram_tensor("y", [CO, H, W], FP32, kind="ExternalOutput")

    with tile.TileContext(nc) as tc, ExitStack() as ctx:
        singles = ctx.enter_context(tc.tile_pool(name="singles", bufs=1))
        work = ctx.enter_context(tc.tile_pool(name="work", bufs=1))
        xstage = ctx.enter_context(tc.tile_pool(name="xstage", bufs=6))
        xtpool = ctx.enter_context(tc.tile_pool(name="xt", bufs=6))

        zrow = singles.tile([128, WP], FP32)
        nc.vector.memset(zrow, 0.0)
        ident = singles.tile([128, 128], FP32)
        make_identity(nc, ident)
        eps_t = singles.tile([128, 1], FP32)
        nc.vector.memset(eps_t, EPS)
        ones_t = singles.tile([128, 1], FP32)
        nc.vector.memset(ones_t, 1.0)

        # ---- small input DMAs (style path) first ----
        wb = singles.tile([128, Z], FP32)
        w_ap = w_d[:]
        nc.sync.dma_start(
            out=wb,
            in_=bass.AP(tensor=w_ap.tensor, offset=w_ap.offset, ap=[[0, 128], [1, Z]]),
        )
        af, ab1 = [], []
        for it in range(IT):
            a = singles.tile([128, Z], FP32, tag=f"af{it}")
            nc.sync.dma_start(out=a, in_=aw_d[it * 128:(it + 1) * 128, :])
            af.append(a)
            abt = singles.tile([128, 1], FP32, tag=f"ab{it}")
            nc.sync.dma_start(
                out=abt, in_=ab_d[it * 128:(it + 1) * 128].rearrange("(p o) -> p o", o=1)
            )
            ab1.append(abt)

        # ---- weight DMA, one chunk tile per (ot, it) so each transpose
        # waits only on its own 0.59MB chunk.  Pool scoped: SBUF reused
        # by og/escr pools below. ----
        wo_ctx = tc.tile_pool(name="wo", bufs=1)
        wopool = wo_ctx.__enter__()
        HALF = (CI // IT) * NKK  # 1152 elements per i-half
        wo = [[None] * IT for _ in range(OT)]

        def load_wo_half(it):
            for ot in range(OT):
                t = wopool.tile([128, HALF], FP32, name=f"wo{ot}{it}",
                                tag=f"wo{ot}{it}")
                wo[ot][it] = t
                nc.sync.dma_start(
                    out=t,
                    in_=wt_d[
                        ot * 128:(ot + 1) * 128, it * 128:(it + 1) * 128
                    ].rearrange("o i kh kw -> o (i kh kw)"),
                )

        # ---- x row-group loads into zero-padded fp32 stage tiles ----
        xg_tiles: dict = {}
        xt_tiles: dict = {}

        def lg_dma(g: int, its=None):
            r0 = g * RG
            lo, hi = r0 - 1, r0 + RG + 1
            clo, chi = max(lo, 0), min(hi, H)
            nrows = chi - clo
            if g not in xg_tiles:
                xg_tiles[g] = [None] * IT
                xt_tiles[g] = []
            for it in its if its is not None else range(IT):
                stg = xstage.tile([128, RG + 2, WP], FP32, name="stg", tag="stg")
                xg_tiles[g][it] = stg
                nc.sync.dma_start(
                    out=stg[:, clo - lo: chi - lo, 1:W + 1],
                    in_=x_d[it * 128:(it + 1) * 128, clo:chi, :],
                )
                nc.gpsimd.tensor_copy(out=stg[:, :, 0], in_=zrow[:, 0:RG + 2])
                nc.gpsimd.tensor_copy(out=stg[:, :, WP - 1], in_=zrow[:, 0:RG + 2])
                if lo < 0:
                    nc.gpsimd.tensor_copy(out=stg[:, 0, :], in_=zrow)
                if hi > H:
                    nc.gpsimd.tensor_copy(out=stg[:, RG + 1, :], in_=zrow)

        # startup DMA order: weights it0, first x half, weights it1, rest
        load_wo_half(0)
        lg_dma(0, its=[0])
        load_wo_half(1)
        lg_dma(0, its=[1])
        lg_dma(1)
        lg_dma(2)

        # ---- style: st = w @ affine_w.T + affine_b + 1 (per i-half) ----
        st, sth = [], []

        def style(it):
            stt = work.tile([128, Z], FP32, name="stt", tag="styletmp")
            nc.vector.tensor_mul(stt, af[it], wb)
            s = singles.tile([128, 1], FP32, name="s", tag=f"st{it}")
            nc.vector.reduce_sum(s, stt, axis=AX.X)
            nc.vector.tensor_add(s, s, ab1[it])
            nc.vector.tensor_scalar_add(s, s, 1.0)
            st.append(s)
            sh = singles.tile([128, 1], FP32, name="sh", tag=f"sth{it}")
            nc.vector.tensor_scalar_mul(sh, s, 0.5)
            sth.append(sh)

        style(0)
        style(1)

        # ---- PE transpose + style-modulated fp16 eviction ----
        # wm[it][i, kk*CO+o] = weight[o, i, kk] * st[i]  (kh==1 slabs get
        # 0.5*st so the winograd build is pure STT).  it0 evictions on DVE,
        # it1 on ACT.
        wm = [
            singles.tile([128, NKK * CO], FP16, name=f"wm{it}", tag=f"wm{it}")
            for it in range(IT)
        ]
        ws = [
            singles.tile([128, 2 * KH], FP16, name=f"ws{it}", tag=f"ws{it}")
            for it in range(IT)
        ]

        def transpose_half(it):
            with tc.tile_pool(name=f"tpsum{it}", bufs=4, space="PSUM") as tps:
                for ot in range(OT):
                    for kk in range(NKK):
                        pt = tps.tile([128, 128], FP32, name="pt", tag="pt")
                        src = wo[ot][it].rearrange("o (i k) -> o i k", k=NKK)[
                            :, :, kk
                        ]
                        nc.tensor.transpose(out=pt, in_=src, identity=ident)
                        dst = wm[it][:, kk * CO + ot * 128: kk * CO + (ot + 1) * 128]
                        scl = sth[it] if kk // KS == 1 else st[it]
                        if it == 0:
                            nc.vector.tensor_scalar_mul(dst, pt, scl)
                        else:
                            nc.scalar.mul(out=dst, in_=pt, mul=scl)

        def wbuild(it):
            # taps: t0 = wm kh0, t1 = 0.5(k0+k2)+k1h, t2 = -0.5(k0+k2)+k1h,
            # t3 = wm kh2  (k1h already holds 0.5*st*k1)
            k0 = wm[it][:, 0 * KH:1 * KH]
            k1h = wm[it][:, 1 * KH:2 * KH]
            k2 = wm[it][:, 2 * KH:3 * KH]
            u1 = work.tile([128, KH], FP16, name="u1", tag=f"u1{it}")
            nc.vector.tensor_add(u1, k0, k2)
            nc.vector.scalar_tensor_tensor(
                ws[it][:, 0:KH], u1, 0.5, k1h, op0=OP.mult, op1=OP.add
            )
            nc.vector.scalar_tensor_tensor(
                ws[it][:, KH:2 * KH], u1, -0.5, k1h, op0=OP.mult, op1=OP.add
            )

        def lhsT(it, t, kw, ot):
            base = kw * CO + ot * 128
            if t == 0:
                return wm[it][:, base:base + 128]
            if t == 3:
                return wm[it][:, 2 * KH + base:2 * KH + base + 128]
            return ws[it][:, (t - 1) * KH + base:(t - 1) * KH + base + 128]

        transpose_half(0)
        wbuild(0)
        transpose_half(1)

        # ---- wsq[it][i, o] = sum_kk wm^2 (x4 for the half-scaled kh1):
        # squares on ACT (scale=2 before Square gives the x4), adds on
        # GpSimd — keeps both off the DVE/PE critical paths.
        wsq = []
        for it in range(IT):
            q = singles.tile([128, CO], FP32, name="wsq", tag=f"wsq{it}")
            nc.scalar.square(q, wm[it][:, 0:CO])
            for kk in range(1, NKK):
                slab = wm[it][:, kk * CO:(kk + 1) * CO]
                tmp = work.tile([128, CO], FP32, name="wsqt",
                                tag=f"wsqt{it}{kk % 2}")
                nc.scalar.activation(
                    out=tmp, in_=slab, func=AF.Square,
                    scale=2.0 if kk // KS == 1 else 1.0,
                )
                nc.gpsimd.tensor_add(q, q, tmp)
            wsq.append(q)

        # ---- x transform: F(2,3) rows, fp32 in / fp16 out on DVE ----
        def transform_it(g: int, it: int):
            xg = xg_tiles[g][it]
            xt = xtpool.tile([128, TH, NP, WP], FP16, name="xt", tag="xt")
            xt_tiles[g].append(xt)
            ev = xg.rearrange("p (r c) w -> p r c w", c=2)  # [128, 9, 2, WP]
            d0 = ev[:, 0:NP, 0, :]   # rows 0,2,..,14
            d1 = ev[:, 0:NP, 1, :]   # rows 1,3,..,15
            d2 = ev[:, 1:NP + 1, 0, :]  # rows 2,4,..,16
            d3 = ev[:, 1:NP + 1, 1, :]  # rows 3,5,..,17
            nc.vector.tensor_sub(xt[:, 0], d0, d2)
            nc.vector.tensor_add(xt[:, 1], d1, d2)
            nc.vector.tensor_sub(xt[:, 2], d1, d2)
            nc.vector.tensor_sub(xt[:, 3], d1, d3)

        transform_it(0, 0)
        wbuild(1)
        transform_it(0, 1)

        # ---- conv machinery ----
        wo_ctx.__exit__(None, None, None)
        opool = ctx.enter_context(tc.tile_pool(name="og", bufs=5))
        spool = ctx.enter_context(tc.tile_pool(name="escr", bufs=2))
        cpsum = ctx.enter_context(tc.tile_pool(name="cpsum", bufs=7, space="PSUM"))
        dpsum = ctx.enter_context(tc.tile_pool(name="dpsum", bufs=1, space="PSUM"))
        dn, ndn = [], []

        def half_matmuls(g: int, ot: int, h: int):
            P = [
                cpsum.tile([128, 512], FP32, name="pg", tag="pg")
                for _ in range(TH)
            ]
            for it in range(IT):
                xt = xt_tiles[g][it]
                for t in range(TH):
                    for kw in range(KS):
                        nc.tensor.matmul(
                            P[t],
                            lhsT=lhsT(it, t, kw, ot),
                            rhs=xt[:, t, 4 * h:4 * h + 4, kw:kw + W],
                            start=(it == 0 and kw == 0),
                            stop=(it == IT - 1 and kw == KS - 1),
                        )
            return P

        def store_og(og, g, ot, h):
            yv = y_d[ot * 128:(ot + 1) * 128].rearrange(
                "o (r j) w -> o r j w", j=2
            )
            r0 = g * (RG // 2) + h * TH
            for j in range(2):
                nc.sync.dma_start(
                    out=yv[:, r0:r0 + TH, j, :], in_=og[:, j]
                )

        def evict_raw(P):
            # all-DVE raw eviction (no dn): used for group 0, demod deferred
            Pv = [p.rearrange("p (a b) -> p a b", b=W) for p in P]
            og = opool.tile([128, 2, TH, W], FP32, name="og", tag="og")
            c1 = spool.tile([128, TH, W], FP32, name="c1", tag="c1")
            u = spool.tile([128, TH, W], FP32, name="u", tag="u")
            v = spool.tile([128, TH, W], FP32, name="v", tag="v")
            nc.vector.tensor_copy(c1, Pv[1])
            nc.vector.tensor_add(u, c1, Pv[0])
            nc.vector.tensor_add(og[:, 0], u, Pv[2])
            nc.vector.tensor_sub(v, c1, Pv[2])
            nc.vector.tensor_sub(og[:, 1], v, Pv[3])
            return og

        def evict(g: int, ot: int, h: int, shallow=False):
            # y0 = (P0+P1+P2)*dn, y1 = (P1-P2-P3)*dn, demod fused:
            # ACT copies P1,P2 off PSUM scaled by dn; DVE STT scales P0/P3
            # on the fly; GpSimd does the one all-SBUF add (non-shallow).
            P = half_matmuls(g, ot, h)
            Pv = [p.rearrange("p (a b) -> p a b", b=W) for p in P]
            og = opool.tile([128, 2, TH, W], FP32, name="og", tag="og")
            c1 = spool.tile([128, TH, W], FP32, name="c1", tag="c1")
            u = spool.tile([128, TH, W], FP32, name="u", tag="u")
            v = spool.tile([128, TH, W], FP32, name="v", tag="v")
            nc.scalar.mul(out=c1, in_=Pv[1], mul=dn[ot])
            nc.vector.scalar_tensor_tensor(
                u, Pv[0], dn[ot], c1, op0=OP.mult, op1=OP.add
            )
            if shallow:
                nc.vector.scalar_tensor_tensor(
                    og[:, 0], Pv[2], dn[ot], u, op0=OP.mult, op1=OP.add
                )
                nc.vector.scalar_tensor_tensor(
                    v, Pv[2], ndn[ot], c1, op0=OP.mult, op1=OP.add
                )
            else:
                c2 = spool.tile([128, TH, W], FP32, name="c2", tag="c2")
                nc.scalar.mul(out=c2, in_=Pv[2], mul=dn[ot])
                nc.gpsimd.tensor_add(og[:, 0], u, c2)
                nc.vector.tensor_sub(v, c1, c2)
            nc.vector.scalar_tensor_tensor(
                og[:, 1], Pv[3], ndn[ot], v, op0=OP.mult, op1=OP.add
            )
            store_og(og, g, ot, h)

        # ---- group 0: conv + raw evict, transforms for g1 interleaved;
        # denominators after g0's matmuls (PE FIFO), demod deferred ----
        ogs = []
        P = half_matmuls(0, 0, 0)
        ogs.append((evict_raw(P), 0, 0))
        P = half_matmuls(0, 0, 1)
        ogs.append((evict_raw(P), 0, 1))
        transform_it(1, 0)
        P = half_matmuls(0, 1, 0)
        ogs.append((evict_raw(P), 1, 0))
        transform_it(1, 1)
        P = half_matmuls(0, 1, 1)
        ogs.append((evict_raw(P), 1, 1))

        def emit_denom():
            for ot in range(OT):
                pd = dpsum.tile([128, 1], FP32, name="pd", tag="pd")
                for it in range(IT):
                    nc.tensor.matmul(
                        pd,
                        lhsT=wsq[it][:, ot * 128:(ot + 1) * 128],
                        rhs=ones_t,
                        start=(it == 0),
                        stop=(it == IT - 1),
                    )
                dcol = singles.tile([128, 1], FP32, name="dn", tag=f"dn{ot}")
                nc.scalar.activation(out=dcol, in_=pd, func=AF.Sqrt, bias=eps_t)
                nc.vector.reciprocal(dcol, dcol)
                dn.append(dcol)
                ncol = singles.tile([128, 1], FP32, name="ndn", tag=f"ndn{ot}")
                nc.vector.tensor_scalar_mul(ncol, dcol, -1.0)
                ndn.append(ncol)

        emit_denom()
        for og, ot, h in ogs:
            nc.scalar.mul(out=og, in_=og, mul=dn[ot])
            store_og(og, 0, ot, h)

        for g in range(1, G):
            if g + 2 < G:
                lg_dma(g + 2)
            last = g == G - 1
            evict(g, 0, 0)
            evict(g, 0, 1)
            if g + 1 < G:
                transform_it(g + 1, 0)
            evict(g, 1, 0, shallow=last)
            if g + 1 < G:
                transform_it(g + 1, 1)
            evict(g, 1, 1, shallow=last)
    nc.finalize()
    return nc


_CACHE: dict = {}


def _get_nc() -> bass.Bass:
    if "nc" not in _CACHE:
        _CACHE["nc"] = build_nc()
    return _CACHE["nc"]


def make_in_maps(x, w, weight, affine_w, affine_b):
    x = np.ascontiguousarray(x, dtype=np.float32)
    w = np.ascontiguousarray(w, dtype=np.float32)
    weight = np.ascontiguousarray(weight, dtype=np.float32)
    affine_w = np.ascontiguousarray(affine_w, dtype=np.float32)
    affine_b = np.ascontiguousarray(affine_b, dtype=np.float32)
    return [
        {
            "x": x[c],
            "w": w[c],
            "weight": weight,
            "affine_w": affine_w,
            "affine_b": affine_b,
        }
        for c in range(B)
    ]


def run_on_hw(inputs: dict, trace: bool = False, tmpdir: str | None = None):
    from concourse.bass_utils import run_bass_kernel_spmd

    nc = _get_nc()
    in_maps = make_in_maps(**inputs)
    res = run_bass_kernel_spmd(
        nc, in_maps, core_ids=list(range(B)), trace=trace, tmpdir=tmpdir
    )
    y = np.stack([r["y"] for r in res.results], axis=0)
    return y, res


def kernel(x, w, weight, affine_w, affine_b):
    y, _ = run_on_hw(
        dict(x=x, w=w, weight=weight, affine_w=affine_w, affine_b=affine_b)
    )
    return y
